# revision 8
# baseline (speedup 1.0000x reference)
"""Trainium2 Bass kernel for additive (Bahdanau-style) attention.

Reference computation (per batch b):
    qp = q @ W1.T                      # (h,)
    vp = v @ W2.T + b2                 # (n, h)
    h  = tanh(qp + vp)                 # (n, h)
    score = h @ Vw.T                   # (n,)
    attn = softmax(where(mask, score, -1e9))
    att_out = attn @ v                 # (v,)

Sharding: data-parallel over batch. 32 batches / 8 cores = 4 per core.
Weights tiny, replicated (passed pre-transposed / bf16-cast from host --
pure layout prep, no FLOPs moved off-device).

Device algorithm per core (B=4 local batches, N=4096, V=H=K=512):
  - Stream v[b] from HBM once, casting f32->bf16 in the SWDGE DMA.
  - DMA-xbar-transpose each [128n x 512v] stage tile into vT[128p, 4a, NT i, 128f]
    where partition p of k-tile a holds v-index a*128+p, free (i,f) is n.
  - PE: vpT[h, n] = W2T.T @ vT in bf16 (4 h-tiles x 4 k-tiles per 512-n chunk).
  - ACT: tanh(vp + (qp+b2)) fused via per-partition bias, output bf16.
  - PE: score chunk [1, 512] = VwT.T @ tanh, accumulated over 4 h-tiles.
  - ACT: exp(score) (softmax without max-subtraction: |score| <= sum|Vw| ~ 55,
    so exp cannot overflow fp32; masked lanes use exp(s)*mask == where-semantics).
  - DVE: S = sum(exp*mask); attn = exp*mask/S; att_out[v] = sum_n attn[n]*vT[v,n]
    via fused tensor_tensor_reduce against a partition-broadcast of attn.
"""

from contextlib import ExitStack

import numpy as np
import ml_dtypes

import concourse.bass as bass
import concourse.tile as tile
import concourse.mybir as mybir
from concourse import bacc
from concourse.bass_utils import run_bass_kernel_spmd

BZ = 32
N_CORES = 8
B = BZ // N_CORES          # batches per core
N = 4096                   # n_step
V = 512                    # v_size
H = 512                    # hidden
K = 512                    # k_size
CHN = 512                  # n per score chunk
HT = H // 128              # 4 h-tiles
VT = V // 128              # 4 v(k)-tiles
KT = K // 128              # 4 k-tiles (q projection)

F32 = mybir.dt.float32
BF16 = mybir.dt.bfloat16
AF = mybir.ActivationFunctionType
OP = mybir.AluOpType

_cache = {}


def _build(b_sz=B, n_sz=N):
    nch = n_sz // CHN          # score chunks per batch
    nt = n_sz // 128           # n-tiles per batch

    nc = bacc.Bacc("TRN2", target_bir_lowering=False, debug=False)

    v_d = nc.dram_tensor("v", [b_sz, n_sz, V], F32, kind="ExternalInput").ap()
    qT_d = nc.dram_tensor("qT", [K, b_sz], BF16, kind="ExternalInput").ap()
    w1T_d = nc.dram_tensor("W1T", [K, H], BF16, kind="ExternalInput").ap()
    w2T_d = nc.dram_tensor("W2T", [V, H], BF16, kind="ExternalInput").ap()
    vwT_d = nc.dram_tensor("VwT", [H, 1], BF16, kind="ExternalInput").ap()
    b2_d = nc.dram_tensor("b2", [H], F32, kind="ExternalInput").ap()
    maskf_d = nc.dram_tensor("maskf", [b_sz, n_sz], BF16, kind="ExternalInput").ap()
    ao_d = nc.dram_tensor("att_out", [b_sz, V], F32, kind="ExternalOutput").ap()
    ad_d = nc.dram_tensor("attn_dist", [b_sz, n_sz], F32, kind="ExternalOutput").ap()

    with tile.TileContext(nc) as tc, ExitStack() as ctx:
        consts = ctx.enter_context(tc.tile_pool(name="consts", bufs=1))
        vt_pool = ctx.enter_context(tc.tile_pool(name="vt", bufs=2))
        stage_pool = ctx.enter_context(tc.tile_pool(name="stage", bufs=4))
        th_pool = ctx.enter_context(tc.tile_pool(name="th", bufs=2))
        rows_pool = ctx.enter_context(tc.tile_pool(name="rows", bufs=2))
        bc_pool = ctx.enter_context(tc.tile_pool(name="bc", bufs=2))
        scr_pool = ctx.enter_context(tc.tile_pool(name="scr", bufs=1))
        ps_pool = ctx.enter_context(tc.tile_pool(name="psum", bufs=2, space="PSUM"))
        ps_qp = ctx.enter_context(tc.tile_pool(name="psumqp", bufs=1, space="PSUM"))

        # ---- load constants ----
        w1T_sb = consts.tile([128, KT, H], BF16)
        nc.sync.dma_start(out=w1T_sb, in_=w1T_d.rearrange("(t p) h -> p t h", p=128))
        w2T_sb = consts.tile([128, VT, H], BF16)
        nc.sync.dma_start(out=w2T_sb, in_=w2T_d.rearrange("(t p) h -> p t h", p=128))
        vwT_sb = consts.tile([128, HT, 1], BF16)
        nc.sync.dma_start(out=vwT_sb, in_=vwT_d.rearrange("(t p) o -> p t o", p=128))
        qT_sb = consts.tile([128, KT, b_sz], BF16)
        nc.sync.dma_start(out=qT_sb, in_=qT_d.rearrange("(t p) b -> p t b", p=128))
        b2_sb = consts.tile([128, HT], F32)
        nc.sync.dma_start(out=b2_sb, in_=b2_d.rearrange("(t p) -> p t", p=128))

        # ---- qp = W1 @ q.T + b2, laid out [128h, h-tile, b] ----
        qpb_sb = consts.tile([128, HT, b_sz], F32)
        for m in range(HT):
            pq = ps_qp.tile([128, b_sz], F32, tag="qp")
            for t in range(KT):
                nc.tensor.matmul(
                    pq,
                    w1T_sb[:, t, m * 128:(m + 1) * 128],
                    qT_sb[:, t, :],
                    start=(t == 0),
                    stop=(t == KT - 1),
                )
            nc.scalar.activation(
                out=qpb_sb[:, m, :], in_=pq, func=AF.Identity,
                bias=b2_sb[:, m:m + 1], scale=1.0,
            )

        for b in range(b_sz):
            # ---- load + transpose v[b] ----
            vT = vt_pool.tile([128, VT, nt, 128], BF16, tag="vt")
            for i in range(nt):
                stg = stage_pool.tile([128, V], BF16, tag="stg")
                nc.gpsimd.dma_start(out=stg, in_=v_d[b, i * 128:(i + 1) * 128, :])
                nc.sync.dma_start(out=vT[:, :, i, :], in_=stg, transpose=True)

            exps = rows_pool.tile([1, n_sz], BF16, tag="exps")
            for c in range(nch):
                # ---- vp + tanh per h-tile ----
                th = th_pool.tile([128, HT, CHN], BF16, tag="th")
                for m in range(HT):
                    ps = ps_pool.tile([128, CHN], F32, tag="vp")
                    for a in range(VT):
                        nc.tensor.matmul(
                            ps,
                            w2T_sb[:, a, m * 128:(m + 1) * 128],
                            vT[:, a, c * 4:(c + 1) * 4, :],
                            start=(a == 0),
                            stop=(a == VT - 1),
                        )
                    nc.scalar.activation(
                        out=th[:, m, :], in_=ps, func=AF.Tanh,
                        bias=qpb_sb[:, m, b:b + 1], scale=1.0,
                    )
                # ---- score chunk ----
                psc = ps_pool.tile([1, CHN], F32, tag="score")
                for m in range(HT):
                    nc.tensor.matmul(
                        psc,
                        vwT_sb[:, m, :],
                        th[:, m, :],
                        start=(m == 0),
                        stop=(m == HT - 1),
                    )
                nc.scalar.activation(
                    out=exps[0:1, c * CHN:(c + 1) * CHN], in_=psc, func=AF.Exp,
                )

            # ---- softmax (no max-subtraction needed; see module docstring) ----
            maskb = rows_pool.tile([1, n_sz], BF16, tag="mask")
            nc.sync.dma_start(out=maskb, in_=maskf_d[b:b + 1, :])
            masked = rows_pool.tile([1, n_sz], BF16, tag="masked")
            ssum = rows_pool.tile([1, 1], F32, tag="ssum")
            nc.vector.scalar_tensor_tensor(
                out=masked, in0=exps, scalar=1.0, in1=maskb,
                op0=OP.mult, op1=OP.mult, accum_out=ssum,
            )
            rec = rows_pool.tile([1, 1], F32, tag="rec")
            nc.vector.reciprocal(rec, ssum)
            attn = rows_pool.tile([1, n_sz], BF16, tag="attn")
            nc.vector.tensor_scalar_mul(attn, masked, rec)
            # bf16 -> f32 cast during the output DMA (SWDGE)
            nc.gpsimd.dma_start(out=ad_d[b:b + 1, :], in_=attn)

            # ---- att_out[v] = sum_n attn[n] * vT[v, n] on DVE ----
            attn_bc = bc_pool.tile([128, n_sz], BF16, tag="bc")
            nc.gpsimd.partition_broadcast(attn_bc, attn)
            ao = rows_pool.tile([128, VT], F32, tag="ao")
            scr = scr_pool.tile([128, n_sz], BF16, tag="scr")
            for a in range(VT):
                nc.vector.scalar_tensor_tensor(
                    out=scr, in0=vT[:, a, :, :], scalar=1.0, in1=attn_bc,
                    op0=OP.mult, op1=OP.mult, accum_out=ao[:, a:a + 1],
                )
            nc.sync.dma_start(
                out=ao_d[b].rearrange("(a p) -> p a", p=128), in_=ao
            )

    nc.compile()
    return nc


def _get_nc():
    if "nc" not in _cache:
        _cache["nc"] = _build()
    return _cache["nc"]


def make_in_map(q, v, mask, W1, W2, b2, Vw):
    """Build one core's input map from that core's batch shard (q, v, mask)."""
    bf = ml_dtypes.bfloat16
    return {
        "v": np.ascontiguousarray(v).astype(np.float32),
        "qT": np.ascontiguousarray(q.T).astype(bf),
        "maskf": mask.astype(bf),
        "W1T": np.ascontiguousarray(W1.T).astype(bf),
        "W2T": np.ascontiguousarray(W2.T).astype(bf),
        "VwT": np.ascontiguousarray(Vw.reshape(1, -1).T).astype(bf),
        "b2": np.ascontiguousarray(b2).astype(np.float32),
    }


def run(q, v, mask, W1, W2, b2, Vw, trace=False, **trace_kwargs):
    nc = _get_nc()
    maps = [
        make_in_map(q[i * B:(i + 1) * B], v[i * B:(i + 1) * B],
                    mask[i * B:(i + 1) * B], W1, W2, b2, Vw)
        for i in range(N_CORES)
    ]
    res = run_bass_kernel_spmd(
        nc, maps, core_ids=list(range(N_CORES)), trace=trace, **trace_kwargs
    )
    att_out = np.concatenate(
        [np.asarray(res.results[i]["att_out"]) for i in range(N_CORES)], axis=0
    ).astype(np.float32)
    attn_dist = np.concatenate(
        [np.asarray(res.results[i]["attn_dist"]) for i in range(N_CORES)], axis=0
    ).astype(np.float32)
    return (att_out, attn_dist), res


def kernel(q, v, mask, W1, W2, b2, Vw):
    (att_out, attn_dist), _ = run(
        np.asarray(q), np.asarray(v), np.asarray(mask),
        np.asarray(W1), np.asarray(W2), np.asarray(b2), np.asarray(Vw),
    )
    return att_out, attn_dist
